# revision 22
# baseline (speedup 1.0000x reference)
"""Cross-attention layer on 8 TRN2 NeuronCores.

Sharding: core i -> (batch b = i//2, head-group g = i%2); each core computes
its head-group's contribution to out[b] through Wo; the host sums the two
partial products per batch (row-split of Wo => partial-sum reduction).

Device kernel works in transposed layout ([channels, tokens]) so the softmax
reduction is along the matmul partition axis (all matmuls bf16, fp32 PSUM):
  Q^T = Wq_g^T x^T, K^T = Wk_g^T ctx^T
  scores^T_h = K_h Q_h^T                 (contraction over head_dim=64)
  E = exp(scores^T/32)                   (ACT exp, no max subtraction)
  E *= mask^T                            (one whole-unit DVE mult)
  V' = [ctx Wv_g | ones(64)]             (ones block -> PE broadcasts the
                                          softmax denominator for free)
  U = V'^T E
  O^T = U[0:64] * recip(U[64:128])       (DVE reciprocal_approx_fast on an
                                          SBUF copy; no ACT ln/exp round-trip)
  out_partial = O^T^T Wo_g               (bf16 out; host adds pairs in fp32)

The PE's 262144 matmul output columns are ~109us at the 2.4 GHz max p-state
but ~218us at the 1.2 GHz mid state, and the Tensor engine only reaches max
after ~3us of gap-free execution -- so the schedule exists to keep the PE
queue stall-free: warmup matmuls ramp the clock while input DMAs land, and
every exp-gated scores group is followed by an independent "filler" chunk of
projection / V-projection / PV matmuls (64 chunks for 64 gaps) so the PE
never waits on the ACT exp stream's PSUM-bank recycling.
"""

import os
import numpy as np
import ml_dtypes

import concourse.mybir as mybir
from concourse import bacc
import concourse.tile as tile
from concourse.bass_utils import run_bass_kernel_spmd

B, T, TC = 4, 1024, 1024
C, CTX_C, H = 1024, 1024, 16
HD = C // H            # 64
P = 128
NCORES = 8
HG = 2                 # head groups
HPG = H // HG          # 8 heads per core
CG = HPG * HD          # 512 channels per group
NT = 512               # matmul moving-dim chunk
KO = C // P            # 8 contraction tiles for projections
MQ = CG // P           # 4 m-groups of Q^T/K^T
SO = TC // P           # 8 s-tiles
T2 = T // NT           # 2 t-chunks
KP = CG // P           # 4 contraction tiles for the out projection
NU = HPG * T2          # 16 units
F32 = mybir.dt.float32
BF16 = mybir.dt.bfloat16
ALU = mybir.AluOpType
ACTF = mybir.ActivationFunctionType

_CACHED_NC = None


def _ensure_ntff_hook():
    """Register the axon NTFF profiling hook if the image's antenv lacks it."""
    try:
        from antenv.axon_hooks import get_axon_ntff_profile_hook  # noqa: F401
        return
    except ImportError:
        pass
    import sys
    import types
    try:
        from trn_agent_boot.trn_boot import _ntff_profile_via_ctypes
        hook = _ntff_profile_via_ctypes("/opt/axon/libaxon_pjrt.so")
    except Exception:
        hook = None
    mod = types.ModuleType("antenv.axon_hooks")
    mod.get_axon_ntff_profile_hook = lambda: hook
    mod.set_axon_ntff_profile_hook = lambda h: None
    sys.modules["antenv.axon_hooks"] = mod
    import antenv
    antenv.axon_hooks = mod


def _hp(h):
    """Partition slice of local head h inside a [128, MQ, ...] channel tile."""
    lo = (h % 2) * HD
    return slice(lo, lo + HD)


def _build_program():
    nc = bacc.Bacc("TRN2", target_bir_lowering=False, debug=False,
                   num_devices=NCORES)
    # all inputs arrive pre-tiled in the SBUF layout (partition-major) so
    # each DMA descriptor is a long contiguous run
    xT = nc.dram_tensor("xT", [P, KO, T], BF16, kind="ExternalInput").ap()
    ctxT = nc.dram_tensor("ctxT", [P, KO, TC], BF16,
                          kind="ExternalInput").ap()
    maskT = nc.dram_tensor("maskT", [P, SO, T], BF16,
                           kind="ExternalInput").ap()
    wq = nc.dram_tensor("wq", [MQ, P, KO, P], BF16, kind="ExternalInput").ap()
    wk = nc.dram_tensor("wk", [MQ, P, KO, P], BF16, kind="ExternalInput").ap()
    wv = nc.dram_tensor("wv", [P, KO, CG], BF16, kind="ExternalInput").ap()
    wo = nc.dram_tensor("wo", [P, KP, C], BF16, kind="ExternalInput").ap()
    out = nc.dram_tensor("out", [T, C], BF16, kind="ExternalOutput").ap()

    with tile.TileContext(nc) as tc:
        with (
            tc.tile_pool(name="persist", bufs=1) as persist,
            tc.tile_pool(name="etp", bufs=9) as etp,
            tc.tile_pool(name="recp", bufs=2) as recp,
            tc.tile_pool(name="usb", bufs=2) as usb,
            tc.tile_pool(name="outp", bufs=3) as outp,
            tc.tile_pool(name="pssc", bufs=3, space="PSUM") as pssc,
            tc.tile_pool(name="psu", bufs=2, space="PSUM") as psu_pool,
        ):
            qt_sb = persist.tile([P, MQ, T], BF16)            # Q^T [(h,d), t]
            kt_sb = persist.tile([P, MQ, TC], BF16)           # K^T [(h,d), s]
            vp_sb = persist.tile([P, SO, HPG, P], BF16)       # V' + ones blk
            mask_sb = persist.tile([P, SO, T], BF16)          # mask^T
            ot_sb = persist.tile([P, KP, T], BF16)            # O^T normalized
            wo_sb = persist.tile([P, KP, C], BF16)
            xT_sb = persist.tile([P, KO, T], BF16)
            ctxT_sb = persist.tile([P, KO, TC], BF16)
            wq_sb = persist.tile([P, MQ, KO, P], BF16)
            wk_sb = persist.tile([P, MQ, KO, P], BF16)
            wv_sb = persist.tile([P, KO, CG], BF16)

            nc.gpsimd.memset(vp_sb[:, :, :, HD:P], 1.0)

            # one queue, strict priority order: the critical path (wq, x,
            # wk, ctx) gets the full DMA bandwidth before anything else
            nc.sync.dma_start(wq_sb[:, 0], wq[0])
            for h in range(2):
                nc.sync.dma_start(xT_sb[:, 4 * h:4 * h + 4, :],
                                  xT[:, 4 * h:4 * h + 4, :])
            for h in range(2):
                nc.sync.dma_start(ctxT_sb[:, 4 * h:4 * h + 4, :],
                                  ctxT[:, 4 * h:4 * h + 4, :])
            nc.sync.dma_start(wk_sb[:, 0], wk[0])
            nc.sync.dma_start(wq_sb[:, 1:MQ], wq.rearrange("m p k c -> p m k c")[:, 1:MQ])
            nc.sync.dma_start(wk_sb[:, 1:MQ], wk.rearrange("m p k c -> p m k c")[:, 1:MQ])
            nc.sync.dma_start(wv_sb, wv)
            nc.sync.dma_start(mask_sb, maskT)
            nc.sync.dma_start(wo_sb, wo)

            def warmup():
                # ramp the PE p-state on wq while the x/ctx DMAs land; the
                # junk psum tile is never read and its bank is recycled
                ps = psu_pool.tile([P, NT], F32, tag="pd")
                for r in range(12):
                    nc.tensor.matmul(ps[:, 0:P], wq_sb[:, 0, r % KO, :],
                                     wq_sb[:, 0, (r + 1) % KO, :],
                                     start=(r == 0), stop=(r == 11))

            def proj_chunk(w_sb, src_sb, dst_sb, mg, piece):
                # piece 0..3: quarter of one m-group (4 matmuls), kc-major
                # so pieces 0-1 need only the first half of the src DMA; the
                # psum tile spans both t2 chunks, evicted once after piece 3
                t2 = piece % 2
                if piece == 0:
                    proj_chunk.ps = pssc.tile([P, 2 * NT], F32, tag="ps",
                                              name="psproj")
                ps = proj_chunk.ps
                for kc in range(4 * (piece // 2), 4 * (piece // 2) + 4):
                    nc.tensor.matmul(
                        ps[:, t2 * NT:(t2 + 1) * NT],
                        w_sb[:, mg, kc, :],
                        src_sb[:, kc, t2 * NT:(t2 + 1) * NT],
                        start=(kc == 0), stop=(kc == KO - 1))
                if piece == 3:
                    nc.vector.tensor_copy(dst_sb[:, mg, :], ps)

            def vproj_chunk(jp, half):
                # half 0/1: one full s-tile (8 matmuls) of so-pair jp
                so = 2 * jp + half
                if half == 0:
                    vproj_chunk.ps = pssc.tile([P, 2 * NT], F32, tag="ps",
                                               name="psv")
                ps = vproj_chunk.ps
                for kc in range(KO):
                    nc.tensor.matmul(
                        ps[:, half * NT:(half + 1) * NT],
                        ctxT_sb[:, kc, so * P:(so + 1) * P],
                        wv_sb[:, kc, :],
                        start=(kc == 0), stop=(kc == KO - 1))
                if half == 1:
                    nc.scalar.activation(
                        vp_sb[:, 2 * jp:2 * jp + 2, :, 0:HD],
                        ps.rearrange("p (s h d) -> p s h d", s=2, h=HPG),
                        ACTF.Copy)

            def pv_chunk(u, piece):
                # piece 0..3: two of the eight PV accumulation matmuls;
                # the DVE normalize chain hangs off piece 3
                h, t2 = u % HPG, u // HPG
                et = et_q[u]
                if piece == 0:
                    pv_chunk.ps = psu_pool.tile([P, NT], F32, tag="pd",
                                                name="pspv")
                psu = pv_chunk.ps
                for so in range(2 * piece, 2 * piece + 2):
                    nc.tensor.matmul(
                        psu, vp_sb[:, so, h, :], et[:, so, :],
                        start=(so == 0), stop=(so == SO - 1))
                if piece == 3:
                    # recip needs a base-0 SBUF input (the isa op mis-handles
                    # partition base shifts), hence the shifted den copy
                    uden = usb.tile([HD, NT], F32, tag="ud")
                    nc.vector.tensor_copy(uden, psu[HD:P, :])
                    rec = recp.tile([HD, NT], F32, tag="rec")
                    nc.vector.reciprocal_approx_fast(rec, uden)
                    nc.vector.tensor_tensor(
                        ot_sb[_hp(h), h // 2, t2 * NT:(t2 + 1) * NT],
                        psu[0:HD, :], rec, ALU.mult)
                    del et_q[u]

            def scores_group(u, et, jp):
                h, t2 = u % HPG, u // HPG
                ps = pssc.tile([P, 2 * NT], F32, tag="ps", name="pssc")
                for i in range(2):
                    so = 2 * jp + i
                    nc.tensor.matmul(
                        ps[:, i * NT:(i + 1) * NT],
                        kt_sb[_hp(h), h // 2, so * P:(so + 1) * P],
                        qt_sb[_hp(h), h // 2, t2 * NT:(t2 + 1) * NT],
                        start=True, stop=True)
                nc.scalar.activation(
                    et[:, 2 * jp:2 * jp + 2, :].rearrange("p a b -> p (a b)"),
                    ps, ACTF.Exp, scale=1.0 / 32.0)

            def mask_unit(u, et):
                t2 = u // HPG
                eng = nc.gpsimd if u < 6 else nc.vector
                eng.tensor_tensor(
                    et, et, mask_sb[:, :, t2 * NT:(t2 + 1) * NT], ALU.mult)

            def dproj_pair(tm):
                # both c2 halves in one 1024-wide pssc tile (free in the
                # tail), single wide eviction + single out DMA
                ps = pssc.tile([P, 2 * NT], F32, tag="ps", name="psdp")
                for c2 in range(2):
                    for kp in range(KP):
                        nc.tensor.matmul(
                            ps[:, c2 * NT:(c2 + 1) * NT],
                            ot_sb[:, kp, tm * P:(tm + 1) * P],
                            wo_sb[:, kp, c2 * NT:(c2 + 1) * NT],
                            start=(kp == 0), stop=(kp == KP - 1))
                o_sb = outp.tile([P, 2 * NT], BF16, tag="ob")
                nc.scalar.activation(o_sb, ps, ACTF.Copy)
                nc.sync.dma_start(out[tm * P:(tm + 1) * P, :], o_sb)

            # ---- software pipeline ----
            # 64 filler chunks, one per scores-group gap:
            #   units 0-5 gaps: q/k projection m-groups 1-3 (4 chunks each)
            #   units 6-7 gaps: V' projection (4 half-chunks per unit)
            #   units 8-15 gaps: PV of units 0-7 (lag 8; frees et for reuse)
            et_q = {}
            fillers = []
            for mg in range(1, MQ):
                for (w, s_, d_) in ((wq_sb, xT_sb, qt_sb),
                                    (wk_sb, ctxT_sb, kt_sb)):
                    for piece in range(4):
                        fillers.append(
                            (lambda w=w, s_=s_, d_=d_, mg=mg, piece=piece:
                             proj_chunk(w, s_, d_, mg, piece)))
            for jp in range(4):
                for half in range(2):
                    fillers.append(
                        (lambda jp=jp, half=half: vproj_chunk(jp, half)))
            for u in range(8):
                for piece in range(4):
                    fillers.append(
                        (lambda u=u, piece=piece: pv_chunk(u, piece)))

            warmup()
            for piece in range(4):
                proj_chunk(wq_sb, xT_sb, qt_sb, 0, piece)
            for piece in range(4):
                proj_chunk(wk_sb, ctxT_sb, kt_sb, 0, piece)

            fi = 0
            for u in range(NU):
                et_q[u] = etp.tile([P, SO, NT], BF16, tag="et",
                                   name=f"et{u}")
                for jp in range(4):
                    scores_group(u, et_q[u], jp)
                    fillers[fi]()
                    fi += 1
                mask_unit(u, et_q[u])
            for u in range(8, 16):
                for piece in range(4):
                    pv_chunk(u, piece)
                if u < 12:
                    dproj_pair(u - 8)   # t2=0 rows, deps done in the middle
            for tm in range(4, 8):
                dproj_pair(tm)
    nc.compile()
    return nc


def _get_program():
    global _CACHED_NC
    if _CACHED_NC is None:
        _CACHED_NC = _build_program()
    return _CACHED_NC


def kernel(x, context, attn_mask, Wq, Wk, Wv, Wo):
    x = np.asarray(x, dtype=np.float32)
    context = np.asarray(context, dtype=np.float32)
    attn_mask = np.asarray(attn_mask)
    Wq = np.asarray(Wq, dtype=np.float32)
    Wk = np.asarray(Wk, dtype=np.float32)
    Wv = np.asarray(Wv, dtype=np.float32)
    Wo = np.asarray(Wo, dtype=np.float32)

    nc = _get_program()
    bf = ml_dtypes.bfloat16
    in_maps = []
    for i in range(NCORES):
        b, g = i // 2, i % 2
        cs = slice(g * CG, (g + 1) * CG)
        def tile_rows(a, n):
            # [n*128, m] -> [128, n, m] with rows r = k*128+p at [p, k]
            return np.ascontiguousarray(
                a.reshape(n, P, -1).transpose(1, 0, 2)).astype(bf)
        in_maps.append({
            "xT": tile_rows(x[b].T, KO),
            "ctxT": tile_rows(context[b].T, KO),
            "maskT": tile_rows(attn_mask[b, 0].T.astype(np.float32), SO),
            "wq": np.stack([tile_rows(Wq[:, cs][:, m * P:(m + 1) * P], KO)
                            for m in range(MQ)]),
            "wk": np.stack([tile_rows(Wk[:, cs][:, m * P:(m + 1) * P], KO)
                            for m in range(MQ)]),
            "wv": tile_rows(Wv[:, cs], KO),
            "wo": tile_rows(Wo[cs, :], KP),
        })

    profile = os.environ.get("KERNEL_PROFILE", "0") == "1"
    if profile:
        _ensure_ntff_hook()
    res = run_bass_kernel_spmd(
        nc, in_maps, list(range(NCORES)),
        trace=profile, trace_cores=[0] if profile else None)
    if profile:
        kernel.last_exec_time_ns = res.exec_time_ns
        kernel.last_trace = res.instructions_and_trace

    out = np.empty((B, T, C), dtype=np.float32)
    for b in range(B):
        out[b] = (res.results[2 * b]["out"].astype(np.float32)
                  + res.results[2 * b + 1]["out"].astype(np.float32))
    return out


# revision 23
# speedup vs baseline: 1.1700x; 1.1700x over previous
"""Cross-attention layer on 8 TRN2 NeuronCores.

Sharding: core i -> (batch b = i//2, head-group g = i%2); each core computes
its head-group's contribution to out[b] through Wo; the host sums the two
partial products per batch (row-split of Wo => partial-sum reduction).

Device kernel works in transposed layout ([channels, tokens]) so the softmax
reduction is along the matmul partition axis (all matmuls bf16, fp32 PSUM):
  Q^T = Wq_g^T x^T, K^T = Wk_g^T ctx^T
  scores^T_h = K_h Q_h^T                 (contraction over head_dim=64)
  E = exp(scores^T/32)                   (ACT exp, no max subtraction)
  E *= mask^T                            (one whole-unit DVE mult)
  V' = [ctx Wv_g | ones(64)]             (ones block -> PE broadcasts the
                                          softmax denominator for free)
  U = V'^T E
  O^T = U[0:64] * recip(U[64:128])       (DVE reciprocal_approx_fast on an
                                          SBUF copy; no ACT ln/exp round-trip)
  out_partial = O^T^T Wo_g               (bf16 out; host adds pairs in fp32)

The PE's 262144 matmul output columns are ~109us at the 2.4 GHz max p-state
but ~218us at the 1.2 GHz mid state, and the Tensor engine only reaches max
after ~3us of gap-free execution -- so the schedule exists to keep the PE
queue stall-free: warmup matmuls ramp the clock while input DMAs land, and
every exp-gated scores group is followed by an independent "filler" chunk of
projection / V-projection / PV matmuls (64 chunks for 64 gaps) so the PE
never waits on the ACT exp stream's PSUM-bank recycling.
"""

import os
import numpy as np
import ml_dtypes

import concourse.mybir as mybir
from concourse import bacc
import concourse.tile as tile
from concourse.bass_utils import run_bass_kernel_spmd

B, T, TC = 4, 1024, 1024
C, CTX_C, H = 1024, 1024, 16
HD = C // H            # 64
P = 128
NCORES = 8
HG = 2                 # head groups
HPG = H // HG          # 8 heads per core
CG = HPG * HD          # 512 channels per group
NT = 512               # matmul moving-dim chunk
KO = C // P            # 8 contraction tiles for projections
MQ = CG // P           # 4 m-groups of Q^T/K^T
SO = TC // P           # 8 s-tiles
T2 = T // NT           # 2 t-chunks
KP = CG // P           # 4 contraction tiles for the out projection
NU = HPG * T2          # 16 units
F32 = mybir.dt.float32
BF16 = mybir.dt.bfloat16
ALU = mybir.AluOpType
ACTF = mybir.ActivationFunctionType

_CACHED_NC = None


def _ensure_ntff_hook():
    """Register the axon NTFF profiling hook if the image's antenv lacks it."""
    try:
        from antenv.axon_hooks import get_axon_ntff_profile_hook  # noqa: F401
        return
    except ImportError:
        pass
    import sys
    import types
    try:
        from trn_agent_boot.trn_boot import _ntff_profile_via_ctypes
        hook = _ntff_profile_via_ctypes("/opt/axon/libaxon_pjrt.so")
    except Exception:
        hook = None
    mod = types.ModuleType("antenv.axon_hooks")
    mod.get_axon_ntff_profile_hook = lambda: hook
    mod.set_axon_ntff_profile_hook = lambda h: None
    sys.modules["antenv.axon_hooks"] = mod
    import antenv
    antenv.axon_hooks = mod


def _hp(h):
    """Partition slice of local head h inside a [128, MQ, ...] channel tile."""
    lo = (h % 2) * HD
    return slice(lo, lo + HD)


def _build_program():
    nc = bacc.Bacc("TRN2", target_bir_lowering=False, debug=False,
                   num_devices=NCORES)
    # all inputs arrive pre-tiled in the SBUF layout (partition-major) so
    # each DMA descriptor is a long contiguous run
    xT = nc.dram_tensor("xT", [P, KO, T], BF16, kind="ExternalInput").ap()
    ctxT = nc.dram_tensor("ctxT", [P, KO, TC], BF16,
                          kind="ExternalInput").ap()
    maskT = nc.dram_tensor("maskT", [P, SO, T], BF16,
                           kind="ExternalInput").ap()
    wq = nc.dram_tensor("wq", [MQ, P, KO, P], BF16, kind="ExternalInput").ap()
    wk = nc.dram_tensor("wk", [MQ, P, KO, P], BF16, kind="ExternalInput").ap()
    wv = nc.dram_tensor("wv", [P, KO, CG], BF16, kind="ExternalInput").ap()
    wo = nc.dram_tensor("wo", [P, KP, C], BF16, kind="ExternalInput").ap()
    out = nc.dram_tensor("out", [T, C], BF16, kind="ExternalOutput").ap()

    with tile.TileContext(nc) as tc:
        with (
            tc.tile_pool(name="persist", bufs=1) as persist,
            tc.tile_pool(name="etp", bufs=9) as etp,
            tc.tile_pool(name="recp", bufs=2) as recp,
            tc.tile_pool(name="usb", bufs=2) as usb,
            tc.tile_pool(name="outp", bufs=3) as outp,
            tc.tile_pool(name="pssc", bufs=3, space="PSUM") as pssc,
            tc.tile_pool(name="psu", bufs=2, space="PSUM") as psu_pool,
        ):
            qt_sb = persist.tile([P, MQ, T], BF16)            # Q^T [(h,d), t]
            kt_sb = persist.tile([P, MQ, TC], BF16)           # K^T [(h,d), s]
            vp_sb = persist.tile([P, SO, HPG, P], BF16)       # V' + ones blk
            mask_sb = persist.tile([P, SO, T], BF16)          # mask^T
            ot_sb = persist.tile([P, KP, T], BF16)            # O^T normalized
            wo_sb = persist.tile([P, KP, C], BF16)
            xT_sb = persist.tile([P, KO, T], BF16)
            ctxT_sb = persist.tile([P, KO, TC], BF16)
            wq_sb = persist.tile([P, MQ, KO, P], BF16)
            wk_sb = persist.tile([P, MQ, KO, P], BF16)
            wv_sb = persist.tile([P, KO, CG], BF16)

            nc.gpsimd.memset(vp_sb[:, :, :, HD:P], 1.0)

            # one queue, strict priority order: the critical path (wq, x,
            # wk, ctx) gets the full DMA bandwidth before anything else
            nc.sync.dma_start(wq_sb[:, 0], wq[0])
            for h in range(2):
                nc.sync.dma_start(xT_sb[:, 4 * h:4 * h + 4, :],
                                  xT[:, 4 * h:4 * h + 4, :])
            for h in range(2):
                nc.sync.dma_start(ctxT_sb[:, 4 * h:4 * h + 4, :],
                                  ctxT[:, 4 * h:4 * h + 4, :])
            nc.sync.dma_start(wk_sb[:, 0], wk[0])
            nc.sync.dma_start(wq_sb[:, 1:MQ], wq.rearrange("m p k c -> p m k c")[:, 1:MQ])
            nc.sync.dma_start(wk_sb[:, 1:MQ], wk.rearrange("m p k c -> p m k c")[:, 1:MQ])
            nc.sync.dma_start(wv_sb, wv)
            nc.sync.dma_start(mask_sb, maskT)
            nc.sync.dma_start(wo_sb, wo)

            def warmup():
                # ramp the PE p-state on wq while the x/ctx DMAs land; the
                # junk psum tile is never read and its bank is recycled
                ps = psu_pool.tile([P, NT], F32, tag="pd")
                for r in range(12):
                    nc.tensor.matmul(ps[:, 0:P], wq_sb[:, 0, r % KO, :],
                                     wq_sb[:, 0, (r + 1) % KO, :],
                                     start=(r == 0), stop=(r == 11))

            def proj_chunk(w_sb, src_sb, dst_sb, mg, piece):
                # piece 0..3: quarter of one m-group (4 matmuls), kc-major
                # so pieces 0-1 need only the first half of the src DMA; the
                # psum tile spans both t2 chunks, evicted once after piece 3
                t2 = piece % 2
                if piece == 0:
                    proj_chunk.ps = pssc.tile([P, 2 * NT], F32, tag="ps",
                                              name="psproj")
                ps = proj_chunk.ps
                for kc in range(4 * (piece // 2), 4 * (piece // 2) + 4):
                    nc.tensor.matmul(
                        ps[:, t2 * NT:(t2 + 1) * NT],
                        w_sb[:, mg, kc, :],
                        src_sb[:, kc, t2 * NT:(t2 + 1) * NT],
                        start=(kc == 0), stop=(kc == KO - 1))
                if piece == 3:
                    nc.vector.tensor_copy(dst_sb[:, mg, :], ps)

            def vproj_chunk(jp, half):
                # half 0/1: one full s-tile (8 matmuls) of so-pair jp
                so = 2 * jp + half
                if half == 0:
                    vproj_chunk.ps = pssc.tile([P, 2 * NT], F32, tag="ps",
                                               name="psv")
                ps = vproj_chunk.ps
                for kc in range(KO):
                    nc.tensor.matmul(
                        ps[:, half * NT:(half + 1) * NT],
                        ctxT_sb[:, kc, so * P:(so + 1) * P],
                        wv_sb[:, kc, :],
                        start=(kc == 0), stop=(kc == KO - 1))
                if half == 1:
                    nc.scalar.activation(
                        vp_sb[:, 2 * jp:2 * jp + 2, :, 0:HD],
                        ps.rearrange("p (s h d) -> p s h d", s=2, h=HPG),
                        ACTF.Copy)

            def pv_chunk(u, piece):
                # piece 0..3: two of the eight PV accumulation matmuls;
                # the DVE normalize chain hangs off piece 3
                h, t2 = u % HPG, u // HPG
                et = et_q[u]
                if piece == 0:
                    pv_chunk.ps = psu_pool.tile([P, NT], F32, tag="pd",
                                                name="pspv")
                psu = pv_chunk.ps
                for so in range(2 * piece, 2 * piece + 2):
                    nc.tensor.matmul(
                        psu, vp_sb[:, so, h, :], et[:, so, :],
                        start=(so == 0), stop=(so == SO - 1))
                if piece == 3:
                    # recip needs a base-0 SBUF input (the isa op mis-handles
                    # partition base shifts), hence the shifted den copy
                    uden = usb.tile([HD, NT], F32, tag="ud")
                    nc.vector.tensor_copy(uden, psu[HD:P, :])
                    rec = recp.tile([HD, NT], F32, tag="rec")
                    nc.vector.reciprocal_approx_fast(rec, uden)
                    nc.vector.tensor_tensor(
                        ot_sb[_hp(h), h // 2, t2 * NT:(t2 + 1) * NT],
                        psu[0:HD, :], rec, ALU.mult)
                    del et_q[u]

            def scores_group(u, et, jp):
                h, t2 = u % HPG, u // HPG
                ps = pssc.tile([P, 2 * NT], F32, tag="ps", name="pssc")
                for i in range(2):
                    so = 2 * jp + i
                    nc.tensor.matmul(
                        ps[:, i * NT:(i + 1) * NT],
                        kt_sb[_hp(h), h // 2, so * P:(so + 1) * P],
                        qt_sb[_hp(h), h // 2, t2 * NT:(t2 + 1) * NT],
                        start=True, stop=True)
                nc.scalar.activation(
                    et[:, 2 * jp:2 * jp + 2, :].rearrange("p a b -> p (a b)"),
                    ps, ACTF.Exp, scale=1.0 / 32.0)

            def mask_unit(u, et):
                t2 = u // HPG
                nc.vector.tensor_tensor(
                    et, et, mask_sb[:, :, t2 * NT:(t2 + 1) * NT], ALU.mult)

            def dproj_pair(tm):
                # both c2 halves in one 1024-wide pssc tile (free in the
                # tail), single wide eviction + single out DMA
                ps = pssc.tile([P, 2 * NT], F32, tag="ps", name="psdp")
                for c2 in range(2):
                    for kp in range(KP):
                        nc.tensor.matmul(
                            ps[:, c2 * NT:(c2 + 1) * NT],
                            ot_sb[:, kp, tm * P:(tm + 1) * P],
                            wo_sb[:, kp, c2 * NT:(c2 + 1) * NT],
                            start=(kp == 0), stop=(kp == KP - 1))
                o_sb = outp.tile([P, 2 * NT], BF16, tag="ob")
                nc.scalar.activation(o_sb, ps, ACTF.Copy)
                nc.sync.dma_start(out[tm * P:(tm + 1) * P, :], o_sb)

            # ---- software pipeline ----
            # 64 filler chunks, one per scores-group gap:
            #   units 0-5 gaps: q/k projection m-groups 1-3 (4 chunks each)
            #   units 6-7 gaps: V' projection (4 half-chunks per unit)
            #   units 8-15 gaps: PV of units 0-7 (lag 8; frees et for reuse)
            et_q = {}
            fillers = []
            for mg in range(1, MQ):
                for (w, s_, d_) in ((wq_sb, xT_sb, qt_sb),
                                    (wk_sb, ctxT_sb, kt_sb)):
                    for piece in range(4):
                        fillers.append(
                            (lambda w=w, s_=s_, d_=d_, mg=mg, piece=piece:
                             proj_chunk(w, s_, d_, mg, piece)))
            for jp in range(4):
                for half in range(2):
                    fillers.append(
                        (lambda jp=jp, half=half: vproj_chunk(jp, half)))
            for u in range(8):
                for piece in range(4):
                    fillers.append(
                        (lambda u=u, piece=piece: pv_chunk(u, piece)))

            warmup()
            for piece in range(4):
                proj_chunk(wq_sb, xT_sb, qt_sb, 0, piece)
            for piece in range(4):
                proj_chunk(wk_sb, ctxT_sb, kt_sb, 0, piece)

            fi = 0
            for u in range(NU):
                et_q[u] = etp.tile([P, SO, NT], BF16, tag="et",
                                   name=f"et{u}")
                for jp in range(4):
                    scores_group(u, et_q[u], jp)
                    fillers[fi]()
                    fi += 1
                mask_unit(u, et_q[u])
            for u in range(8, 16):
                for piece in range(4):
                    pv_chunk(u, piece)
                if u < 12:
                    dproj_pair(u - 8)   # t2=0 rows, deps done in the middle
            for tm in range(4, 8):
                dproj_pair(tm)
    nc.compile()
    return nc


def _get_program():
    global _CACHED_NC
    if _CACHED_NC is None:
        _CACHED_NC = _build_program()
    return _CACHED_NC


def kernel(x, context, attn_mask, Wq, Wk, Wv, Wo):
    x = np.asarray(x, dtype=np.float32)
    context = np.asarray(context, dtype=np.float32)
    attn_mask = np.asarray(attn_mask)
    Wq = np.asarray(Wq, dtype=np.float32)
    Wk = np.asarray(Wk, dtype=np.float32)
    Wv = np.asarray(Wv, dtype=np.float32)
    Wo = np.asarray(Wo, dtype=np.float32)

    nc = _get_program()
    bf = ml_dtypes.bfloat16
    in_maps = []
    for i in range(NCORES):
        b, g = i // 2, i % 2
        cs = slice(g * CG, (g + 1) * CG)
        def tile_rows(a, n):
            # [n*128, m] -> [128, n, m] with rows r = k*128+p at [p, k]
            return np.ascontiguousarray(
                a.reshape(n, P, -1).transpose(1, 0, 2)).astype(bf)
        in_maps.append({
            "xT": tile_rows(x[b].T, KO),
            "ctxT": tile_rows(context[b].T, KO),
            "maskT": tile_rows(attn_mask[b, 0].T.astype(np.float32), SO),
            "wq": np.stack([tile_rows(Wq[:, cs][:, m * P:(m + 1) * P], KO)
                            for m in range(MQ)]),
            "wk": np.stack([tile_rows(Wk[:, cs][:, m * P:(m + 1) * P], KO)
                            for m in range(MQ)]),
            "wv": tile_rows(Wv[:, cs], KO),
            "wo": tile_rows(Wo[cs, :], KP),
        })

    profile = os.environ.get("KERNEL_PROFILE", "0") == "1"
    if profile:
        _ensure_ntff_hook()
    res = run_bass_kernel_spmd(
        nc, in_maps, list(range(NCORES)),
        trace=profile, trace_cores=[0] if profile else None)
    if profile:
        kernel.last_exec_time_ns = res.exec_time_ns
        kernel.last_trace = res.instructions_and_trace

    out = np.empty((B, T, C), dtype=np.float32)
    for b in range(B):
        out[b] = (res.results[2 * b]["out"].astype(np.float32)
                  + res.results[2 * b + 1]["out"].astype(np.float32))
    return out


# revision 24
# speedup vs baseline: 1.1845x; 1.0124x over previous
"""Cross-attention layer on 8 TRN2 NeuronCores.

Sharding: core i -> (batch b = i//2, head-group g = i%2); each core computes
its head-group's contribution to out[b] through Wo; the host sums the two
partial products per batch (row-split of Wo => partial-sum reduction).

Device kernel works in transposed layout ([channels, tokens]) so the softmax
reduction is along the matmul partition axis (all matmuls bf16, fp32 PSUM):
  Q^T = Wq_g^T x^T, K^T = Wk_g^T ctx^T
  scores^T_h = K_h Q_h^T                 (contraction over head_dim=64)
  E = exp(scores^T/32)                   (ACT exp, no max subtraction)
  E *= mask^T                            (one whole-unit DVE mult)
  V' = [ctx Wv_g | ones(64)]             (ones block -> PE broadcasts the
                                          softmax denominator for free)
  U = V'^T E
  O^T = U[0:64] * recip(U[64:128])       (DVE reciprocal_approx_fast on an
                                          SBUF copy; no ACT ln/exp round-trip)
  out_partial = O^T^T Wo_g               (bf16 out; host adds pairs in fp32)

The PE's 262144 matmul output columns are ~109us at the 2.4 GHz max p-state
but ~218us at the 1.2 GHz mid state, and the Tensor engine only reaches max
after ~3us of gap-free execution -- so the schedule exists to keep the PE
queue stall-free: warmup matmuls ramp the clock while input DMAs land, and
every exp-gated scores group is followed by an independent "filler" chunk of
projection / V-projection / PV matmuls (64 chunks for 64 gaps) so the PE
never waits on the ACT exp stream's PSUM-bank recycling.
"""

import os
import numpy as np
import ml_dtypes

import concourse.mybir as mybir
from concourse import bacc
import concourse.tile as tile
from concourse.bass_utils import run_bass_kernel_spmd

B, T, TC = 4, 1024, 1024
C, CTX_C, H = 1024, 1024, 16
HD = C // H            # 64
P = 128
NCORES = 8
HG = 2                 # head groups
HPG = H // HG          # 8 heads per core
CG = HPG * HD          # 512 channels per group
NT = 512               # matmul moving-dim chunk
KO = C // P            # 8 contraction tiles for projections
MQ = CG // P           # 4 m-groups of Q^T/K^T
SO = TC // P           # 8 s-tiles
T2 = T // NT           # 2 t-chunks
KP = CG // P           # 4 contraction tiles for the out projection
NU = HPG * T2          # 16 units
F32 = mybir.dt.float32
BF16 = mybir.dt.bfloat16
ALU = mybir.AluOpType
ACTF = mybir.ActivationFunctionType

_CACHED_NC = None


def _ensure_ntff_hook():
    """Register the axon NTFF profiling hook if the image's antenv lacks it."""
    try:
        from antenv.axon_hooks import get_axon_ntff_profile_hook  # noqa: F401
        return
    except ImportError:
        pass
    import sys
    import types
    try:
        from trn_agent_boot.trn_boot import _ntff_profile_via_ctypes
        hook = _ntff_profile_via_ctypes("/opt/axon/libaxon_pjrt.so")
    except Exception:
        hook = None
    mod = types.ModuleType("antenv.axon_hooks")
    mod.get_axon_ntff_profile_hook = lambda: hook
    mod.set_axon_ntff_profile_hook = lambda h: None
    sys.modules["antenv.axon_hooks"] = mod
    import antenv
    antenv.axon_hooks = mod


def _hp(h):
    """Partition slice of local head h inside a [128, MQ, ...] channel tile."""
    lo = (h % 2) * HD
    return slice(lo, lo + HD)


def _build_program():
    nc = bacc.Bacc("TRN2", target_bir_lowering=False, debug=False,
                   num_devices=NCORES)
    # all inputs arrive pre-tiled in the SBUF layout (partition-major) so
    # each DMA descriptor is a long contiguous run
    xT = nc.dram_tensor("xT", [P, KO, T], BF16, kind="ExternalInput").ap()
    ctxT = nc.dram_tensor("ctxT", [P, KO, TC], BF16,
                          kind="ExternalInput").ap()
    maskT = nc.dram_tensor("maskT", [P, SO, T], BF16,
                           kind="ExternalInput").ap()
    wq = nc.dram_tensor("wq", [MQ, P, KO, P], BF16, kind="ExternalInput").ap()
    wk = nc.dram_tensor("wk", [MQ, P, KO, P], BF16, kind="ExternalInput").ap()
    wv = nc.dram_tensor("wv", [P, KO, CG], BF16, kind="ExternalInput").ap()
    wo = nc.dram_tensor("wo", [P, KP, C], BF16, kind="ExternalInput").ap()
    out = nc.dram_tensor("out", [T, C], BF16, kind="ExternalOutput").ap()

    with tile.TileContext(nc) as tc:
        with (
            tc.tile_pool(name="persist", bufs=1) as persist,
            tc.tile_pool(name="etp", bufs=9) as etp,
            tc.tile_pool(name="recp", bufs=2) as recp,
            tc.tile_pool(name="usb", bufs=2) as usb,
            tc.tile_pool(name="outp", bufs=3) as outp,
            tc.tile_pool(name="pssc", bufs=3, space="PSUM") as pssc,
            tc.tile_pool(name="psu", bufs=2, space="PSUM") as psu_pool,
        ):
            qt_sb = persist.tile([P, MQ, T], BF16)            # Q^T [(h,d), t]
            kt_sb = persist.tile([P, MQ, TC], BF16)           # K^T [(h,d), s]
            vp_sb = persist.tile([P, SO, HPG, P], BF16)       # V' + ones blk
            mask_sb = persist.tile([P, SO, T], BF16)          # mask^T
            ot_sb = persist.tile([P, KP, T], BF16)            # O^T normalized
            wo_sb = persist.tile([P, KP, C], BF16)
            xT_sb = persist.tile([P, KO, T], BF16)
            ctxT_sb = persist.tile([P, KO, TC], BF16)
            wq_sb = persist.tile([P, MQ, KO, P], BF16)
            wk_sb = persist.tile([P, MQ, KO, P], BF16)
            wv_sb = persist.tile([P, KO, CG], BF16)

            nc.gpsimd.memset(vp_sb[:, :, :, HD:P], 1.0)

            # one queue, strict priority order: the critical path (wq, x,
            # wk, ctx) gets the full DMA bandwidth before anything else
            nc.sync.dma_start(wq_sb[:, 0], wq[0])
            for h in range(2):
                nc.sync.dma_start(xT_sb[:, 4 * h:4 * h + 4, :],
                                  xT[:, 4 * h:4 * h + 4, :])
            for h in range(2):
                nc.sync.dma_start(ctxT_sb[:, 4 * h:4 * h + 4, :],
                                  ctxT[:, 4 * h:4 * h + 4, :])
            nc.sync.dma_start(wk_sb[:, 0], wk[0])
            nc.sync.dma_start(wq_sb[:, 1:MQ], wq.rearrange("m p k c -> p m k c")[:, 1:MQ])
            nc.sync.dma_start(wk_sb[:, 1:MQ], wk.rearrange("m p k c -> p m k c")[:, 1:MQ])
            nc.sync.dma_start(wv_sb, wv)
            nc.sync.dma_start(mask_sb, maskT)
            nc.sync.dma_start(wo_sb, wo)

            def warmup():
                # ramp the PE p-state on wq while the x/ctx DMAs land; the
                # junk psum tile is never read and its bank is recycled
                ps = psu_pool.tile([P, NT], F32, tag="pd")
                for r in range(12):
                    nc.tensor.matmul(ps[:, 0:P], wq_sb[:, 0, r % KO, :],
                                     wq_sb[:, 0, (r + 1) % KO, :],
                                     start=(r == 0), stop=(r == 11))

            def proj_chunk(w_sb, src_sb, dst_sb, mg, piece):
                # piece 0..3: quarter of one m-group (4 matmuls), kc-major
                # so pieces 0-1 need only the first half of the src DMA; the
                # psum tile spans both t2 chunks, evicted once after piece 3
                t2 = piece % 2
                if piece == 0:
                    proj_chunk.ps = pssc.tile([P, 2 * NT], F32, tag="ps",
                                              name="psproj")
                ps = proj_chunk.ps
                for kc in range(4 * (piece // 2), 4 * (piece // 2) + 4):
                    nc.tensor.matmul(
                        ps[:, t2 * NT:(t2 + 1) * NT],
                        w_sb[:, mg, kc, :],
                        src_sb[:, kc, t2 * NT:(t2 + 1) * NT],
                        start=(kc == 0), stop=(kc == KO - 1))
                if piece == 3:
                    nc.vector.tensor_copy(dst_sb[:, mg, :], ps)

            def vproj_chunk(jp, half):
                # half 0/1: one full s-tile (8 matmuls) of so-pair jp
                so = 2 * jp + half
                if half == 0:
                    vproj_chunk.ps = pssc.tile([P, 2 * NT], F32, tag="ps",
                                               name="psv")
                ps = vproj_chunk.ps
                for kc in range(KO):
                    nc.tensor.matmul(
                        ps[:, half * NT:(half + 1) * NT],
                        ctxT_sb[:, kc, so * P:(so + 1) * P],
                        wv_sb[:, kc, :],
                        start=(kc == 0), stop=(kc == KO - 1))
                if half == 1:
                    nc.vector.tensor_copy(
                        vp_sb[:, 2 * jp:2 * jp + 2, :, 0:HD],
                        ps.rearrange("p (s h d) -> p s h d", s=2, h=HPG))

            def pv_chunk(u, piece):
                # piece 0..3: two of the eight PV accumulation matmuls;
                # the DVE normalize chain hangs off piece 3
                h, t2 = u % HPG, u // HPG
                et = et_q[u]
                if piece == 0:
                    pv_chunk.ps = psu_pool.tile([P, NT], F32, tag="pd",
                                                name="pspv")
                psu = pv_chunk.ps
                for so in range(2 * piece, 2 * piece + 2):
                    nc.tensor.matmul(
                        psu, vp_sb[:, so, h, :], et[:, so, :],
                        start=(so == 0), stop=(so == SO - 1))
                if piece == 3:
                    # recip needs a base-0 SBUF input (the isa op mis-handles
                    # partition base shifts), hence the shifted den copy
                    uden = usb.tile([HD, NT], F32, tag="ud")
                    nc.vector.tensor_copy(uden, psu[HD:P, :])
                    rec = recp.tile([HD, NT], F32, tag="rec")
                    nc.vector.reciprocal_approx_fast(rec, uden)
                    nc.vector.tensor_tensor(
                        ot_sb[_hp(h), h // 2, t2 * NT:(t2 + 1) * NT],
                        psu[0:HD, :], rec, ALU.mult)
                    del et_q[u]

            def scores_group(u, et, jp):
                h, t2 = u % HPG, u // HPG
                ps = pssc.tile([P, 2 * NT], F32, tag="ps", name="pssc")
                for i in range(2):
                    so = 2 * jp + i
                    nc.tensor.matmul(
                        ps[:, i * NT:(i + 1) * NT],
                        kt_sb[_hp(h), h // 2, so * P:(so + 1) * P],
                        qt_sb[_hp(h), h // 2, t2 * NT:(t2 + 1) * NT],
                        start=True, stop=True)
                nc.scalar.activation(
                    et[:, 2 * jp:2 * jp + 2, :].rearrange("p a b -> p (a b)"),
                    ps, ACTF.Exp, scale=1.0 / 32.0)

            def mask_unit(u, et):
                t2 = u // HPG
                nc.vector.tensor_tensor(
                    et, et, mask_sb[:, :, t2 * NT:(t2 + 1) * NT], ALU.mult)

            def dproj_pair(tm):
                # both c2 halves in one 1024-wide pssc tile (free in the
                # tail), single wide eviction + single out DMA
                ps = pssc.tile([P, 2 * NT], F32, tag="ps", name="psdp")
                for c2 in range(2):
                    for kp in range(KP):
                        nc.tensor.matmul(
                            ps[:, c2 * NT:(c2 + 1) * NT],
                            ot_sb[:, kp, tm * P:(tm + 1) * P],
                            wo_sb[:, kp, c2 * NT:(c2 + 1) * NT],
                            start=(kp == 0), stop=(kp == KP - 1))
                o_sb = outp.tile([P, 2 * NT], BF16, tag="ob")
                nc.scalar.activation(o_sb, ps, ACTF.Copy)
                nc.sync.dma_start(out[tm * P:(tm + 1) * P, :], o_sb)

            # ---- software pipeline ----
            # 64 filler chunks, one per scores-group gap:
            #   units 0-5 gaps: q/k projection m-groups 1-3 (4 chunks each)
            #   units 6-7 gaps: V' projection (4 half-chunks per unit)
            #   units 8-15 gaps: PV of units 0-7 (lag 8; frees et for reuse)
            et_q = {}
            fillers = []
            for mg in range(1, MQ):
                for (w, s_, d_) in ((wq_sb, xT_sb, qt_sb),
                                    (wk_sb, ctxT_sb, kt_sb)):
                    for piece in range(4):
                        fillers.append(
                            (lambda w=w, s_=s_, d_=d_, mg=mg, piece=piece:
                             proj_chunk(w, s_, d_, mg, piece)))
            for jp in range(4):
                for half in range(2):
                    fillers.append(
                        (lambda jp=jp, half=half: vproj_chunk(jp, half)))
            for u in range(8):
                for piece in range(4):
                    fillers.append(
                        (lambda u=u, piece=piece: pv_chunk(u, piece)))

            warmup()
            for piece in range(4):
                proj_chunk(wq_sb, xT_sb, qt_sb, 0, piece)
            for piece in range(4):
                proj_chunk(wk_sb, ctxT_sb, kt_sb, 0, piece)

            fi = 0
            for u in range(NU):
                et_q[u] = etp.tile([P, SO, NT], BF16, tag="et",
                                   name=f"et{u}")
                for jp in range(4):
                    scores_group(u, et_q[u], jp)
                    fillers[fi]()
                    fi += 1
                mask_unit(u, et_q[u])
            for u in range(8, 16):
                for piece in range(4):
                    pv_chunk(u, piece)
                if u < 12:
                    dproj_pair(u - 8)   # t2=0 rows, deps done in the middle
            for tm in range(4, 8):
                dproj_pair(tm)
    nc.compile()
    return nc


def _get_program():
    global _CACHED_NC
    if _CACHED_NC is None:
        _CACHED_NC = _build_program()
    return _CACHED_NC


def kernel(x, context, attn_mask, Wq, Wk, Wv, Wo):
    x = np.asarray(x, dtype=np.float32)
    context = np.asarray(context, dtype=np.float32)
    attn_mask = np.asarray(attn_mask)
    Wq = np.asarray(Wq, dtype=np.float32)
    Wk = np.asarray(Wk, dtype=np.float32)
    Wv = np.asarray(Wv, dtype=np.float32)
    Wo = np.asarray(Wo, dtype=np.float32)

    nc = _get_program()
    bf = ml_dtypes.bfloat16
    in_maps = []
    for i in range(NCORES):
        b, g = i // 2, i % 2
        cs = slice(g * CG, (g + 1) * CG)
        def tile_rows(a, n):
            # [n*128, m] -> [128, n, m] with rows r = k*128+p at [p, k]
            return np.ascontiguousarray(
                a.reshape(n, P, -1).transpose(1, 0, 2)).astype(bf)
        in_maps.append({
            "xT": tile_rows(x[b].T, KO),
            "ctxT": tile_rows(context[b].T, KO),
            "maskT": tile_rows(attn_mask[b, 0].T.astype(np.float32), SO),
            "wq": np.stack([tile_rows(Wq[:, cs][:, m * P:(m + 1) * P], KO)
                            for m in range(MQ)]),
            "wk": np.stack([tile_rows(Wk[:, cs][:, m * P:(m + 1) * P], KO)
                            for m in range(MQ)]),
            "wv": tile_rows(Wv[:, cs], KO),
            "wo": tile_rows(Wo[cs, :], KP),
        })

    profile = os.environ.get("KERNEL_PROFILE", "0") == "1"
    if profile:
        _ensure_ntff_hook()
    res = run_bass_kernel_spmd(
        nc, in_maps, list(range(NCORES)),
        trace=profile, trace_cores=[0] if profile else None)
    if profile:
        kernel.last_exec_time_ns = res.exec_time_ns
        kernel.last_trace = res.instructions_and_trace

    out = np.empty((B, T, C), dtype=np.float32)
    for b in range(B):
        out[b] = (res.results[2 * b]["out"].astype(np.float32)
                  + res.results[2 * b + 1]["out"].astype(np.float32))
    return out


# revision 28
# speedup vs baseline: 1.2231x; 1.0326x over previous
"""Cross-attention layer on 8 TRN2 NeuronCores.

Sharding: core i -> (batch b = i//2, head-group g = i%2); each core computes
its head-group's contribution to out[b] through Wo; the host sums the two
partial products per batch (row-split of Wo => partial-sum reduction).

Device kernel works in transposed layout ([channels, tokens]) so the softmax
reduction is along the matmul partition axis (all matmuls bf16, fp32 PSUM):
  Q^T = Wq_g^T x^T, K^T = Wk_g^T ctx^T
  scores^T_h = K_h Q_h^T                 (contraction over head_dim=64)
  E = exp(scores^T/32)                   (ACT exp, no max subtraction)
  E *= mask^T                            (one whole-unit DVE mult)
  V' = [ctx Wv_g | ones(64)]             (ones block -> PE broadcasts the
                                          softmax denominator for free)
  U = V'^T E
  O^T = U[0:64] * recip(U[64:128])       (DVE reciprocal_approx_fast on an
                                          SBUF copy; no ACT ln/exp round-trip)
  out_partial = O^T^T Wo_g               (bf16 out; host adds pairs in fp32)

The PE's 262144 matmul output columns are ~109us at the 2.4 GHz max p-state
but ~218us at the 1.2 GHz mid state, and the Tensor engine only reaches max
after ~3us of gap-free execution -- so the schedule exists to keep the PE
queue stall-free: warmup matmuls ramp the clock while input DMAs land, and
every exp-gated scores group is followed by an independent "filler" chunk of
projection / V-projection / PV matmuls (64 chunks for 64 gaps) so the PE
never waits on the ACT exp stream's PSUM-bank recycling.
"""

import os
import numpy as np
import ml_dtypes

import concourse.mybir as mybir
from concourse import bacc
import concourse.tile as tile
from concourse.bass_utils import run_bass_kernel_spmd

B, T, TC = 4, 1024, 1024
C, CTX_C, H = 1024, 1024, 16
HD = C // H            # 64
P = 128
NCORES = 8
HG = 2                 # head groups
HPG = H // HG          # 8 heads per core
CG = HPG * HD          # 512 channels per group
NT = 512               # matmul moving-dim chunk
KO = C // P            # 8 contraction tiles for projections
MQ = CG // P           # 4 m-groups of Q^T/K^T
SO = TC // P           # 8 s-tiles
T2 = T // NT           # 2 t-chunks
KP = CG // P           # 4 contraction tiles for the out projection
NU = HPG * T2          # 16 units
F32 = mybir.dt.float32
BF16 = mybir.dt.bfloat16
ALU = mybir.AluOpType
ACTF = mybir.ActivationFunctionType

_CACHED_NC = None


def _ensure_ntff_hook():
    """Register the axon NTFF profiling hook if the image's antenv lacks it."""
    try:
        from antenv.axon_hooks import get_axon_ntff_profile_hook  # noqa: F401
        return
    except ImportError:
        pass
    import sys
    import types
    try:
        from trn_agent_boot.trn_boot import _ntff_profile_via_ctypes
        hook = _ntff_profile_via_ctypes("/opt/axon/libaxon_pjrt.so")
    except Exception:
        hook = None
    mod = types.ModuleType("antenv.axon_hooks")
    mod.get_axon_ntff_profile_hook = lambda: hook
    mod.set_axon_ntff_profile_hook = lambda h: None
    sys.modules["antenv.axon_hooks"] = mod
    import antenv
    antenv.axon_hooks = mod


def _hp(h):
    """Partition slice of local head h inside a [128, MQ, ...] channel tile."""
    lo = (h % 2) * HD
    return slice(lo, lo + HD)


def _build_program():
    nc = bacc.Bacc("TRN2", target_bir_lowering=False, debug=False,
                   num_devices=NCORES)
    # all inputs arrive pre-tiled in the SBUF layout (partition-major) so
    # each DMA descriptor is a long contiguous run
    xT = nc.dram_tensor("xT", [P, KO, T], BF16, kind="ExternalInput").ap()
    ctxT = nc.dram_tensor("ctxT", [P, KO, TC], BF16,
                          kind="ExternalInput").ap()
    maskT = nc.dram_tensor("maskT", [P, SO, T], BF16,
                           kind="ExternalInput").ap()
    wq = nc.dram_tensor("wq", [MQ, P, KO, P], BF16, kind="ExternalInput").ap()
    wk = nc.dram_tensor("wk", [MQ, P, KO, P], BF16, kind="ExternalInput").ap()
    wv = nc.dram_tensor("wv", [P, KO, CG], BF16, kind="ExternalInput").ap()
    wo = nc.dram_tensor("wo", [P, KP, C], BF16, kind="ExternalInput").ap()
    out = nc.dram_tensor("out", [T, C], BF16, kind="ExternalOutput").ap()

    with tile.TileContext(nc) as tc:
        with (
            tc.tile_pool(name="persist", bufs=1) as persist,
            tc.tile_pool(name="etp", bufs=9) as etp,
            tc.tile_pool(name="recp", bufs=2) as recp,
            tc.tile_pool(name="usb", bufs=2) as usb,
            tc.tile_pool(name="outp", bufs=3) as outp,
            tc.tile_pool(name="pssc", bufs=3, space="PSUM") as pssc,
            tc.tile_pool(name="psu", bufs=2, space="PSUM") as psu_pool,
        ):
            qt_sb = persist.tile([P, MQ, T], BF16)            # Q^T [(h,d), t]
            kt_sb = persist.tile([P, MQ, TC], BF16)           # K^T [(h,d), s]
            vp_sb = persist.tile([P, SO, HPG, P], BF16)       # V' + ones blk
            mask_sb = persist.tile([P, SO, T], BF16)          # mask^T
            ot_sb = persist.tile([P, KP, T], BF16)            # O^T normalized
            wo_sb = persist.tile([P, KP, C], BF16)
            xT_sb = persist.tile([P, KO, T], BF16)
            ctxT_sb = persist.tile([P, KO, TC], BF16)
            wq_sb = persist.tile([P, MQ, KO, P], BF16)
            wk_sb = persist.tile([P, MQ, KO, P], BF16)
            wv_sb = persist.tile([P, KO, CG], BF16)

            nc.gpsimd.memset(vp_sb[:, :, :, HD:P], 1.0)

            # one queue, strict priority order: the critical path (wq, x,
            # wk, ctx) gets the full DMA bandwidth before anything else
            nc.sync.dma_start(wq_sb[:, 0], wq[0])
            for h in range(2):
                nc.sync.dma_start(xT_sb[:, 4 * h:4 * h + 4, :],
                                  xT[:, 4 * h:4 * h + 4, :])
            for h in range(2):
                nc.sync.dma_start(ctxT_sb[:, 4 * h:4 * h + 4, :],
                                  ctxT[:, 4 * h:4 * h + 4, :])
            nc.sync.dma_start(wk_sb[:, 0], wk[0])
            nc.sync.dma_start(wq_sb[:, 1:MQ], wq.rearrange("m p k c -> p m k c")[:, 1:MQ])
            nc.sync.dma_start(wk_sb[:, 1:MQ], wk.rearrange("m p k c -> p m k c")[:, 1:MQ])
            nc.sync.dma_start(wv_sb, wv)
            nc.sync.dma_start(mask_sb, maskT)
            nc.sync.dma_start(wo_sb, wo)

            def warmup():
                # ramp the PE p-state on wq while the x/ctx DMAs land; the
                # junk psum tile is never read and its bank is recycled
                ps = psu_pool.tile([P, NT], F32, tag="pd")
                for r in range(12):
                    nc.tensor.matmul(ps[:, 0:P], wq_sb[:, 0, r % KO, :],
                                     wq_sb[:, 0, (r + 1) % KO, :],
                                     start=(r == 0), stop=(r == 11))

            def proj_chunk(w_sb, src_sb, dst_sb, mg, piece):
                # piece 0..3: quarter of one m-group (4 matmuls), kc-major
                # so pieces 0-1 need only the first half of the src DMA; the
                # psum tile spans both t2 chunks, evicted once after piece 3
                t2 = piece % 2
                if piece == 0:
                    proj_chunk.ps = pssc.tile([P, 2 * NT], F32, tag="ps",
                                              name="psproj")
                ps = proj_chunk.ps
                for kc in range(4 * (piece // 2), 4 * (piece // 2) + 4):
                    nc.tensor.matmul(
                        ps[:, t2 * NT:(t2 + 1) * NT],
                        w_sb[:, mg, kc, :],
                        src_sb[:, kc, t2 * NT:(t2 + 1) * NT],
                        start=(kc == 0), stop=(kc == KO - 1))
                if piece == 3:
                    nc.vector.tensor_copy(dst_sb[:, mg, :], ps)

            def vproj_chunk(jp, half):
                # half 0/1: one full s-tile (8 matmuls) of so-pair jp
                so = 2 * jp + half
                if half == 0:
                    vproj_chunk.ps = pssc.tile([P, 2 * NT], F32, tag="ps",
                                               name="psv")
                ps = vproj_chunk.ps
                for kc in range(KO):
                    nc.tensor.matmul(
                        ps[:, half * NT:(half + 1) * NT],
                        ctxT_sb[:, kc, so * P:(so + 1) * P],
                        wv_sb[:, kc, :],
                        start=(kc == 0), stop=(kc == KO - 1))
                if half == 1:
                    nc.vector.tensor_copy(
                        vp_sb[:, 2 * jp:2 * jp + 2, :, 0:HD],
                        ps.rearrange("p (s h d) -> p s h d", s=2, h=HPG))

            def pv_chunk(u, piece):
                # piece 0..3: two of the eight PV accumulation matmuls;
                # the DVE normalize chain hangs off piece 3
                h, t2 = u % HPG, u // HPG
                et = et_q[u]
                if piece == 0:
                    pv_chunk.ps = psu_pool.tile([P, NT], F32, tag="pd",
                                                name="pspv")
                psu = pv_chunk.ps
                for so in range(2 * piece, 2 * piece + 2):
                    nc.tensor.matmul(
                        psu, vp_sb[:, so, h, :], et[:, so, :],
                        start=(so == 0), stop=(so == SO - 1))
                if piece == 3:
                    # recip needs a base-0 SBUF input (the isa op mis-handles
                    # partition base shifts), hence the shifted den copy
                    uden = usb.tile([HD, NT], F32, tag="ud")
                    nc.vector.tensor_copy(uden, psu[HD:P, :])
                    rec = recp.tile([HD, NT], F32, tag="rec")
                    nc.vector.reciprocal_approx_fast(rec, uden)
                    nc.vector.tensor_tensor(
                        ot_sb[_hp(h), h // 2, t2 * NT:(t2 + 1) * NT],
                        psu[0:HD, :], rec, ALU.mult)
                    del et_q[u]

            def scores_group(u, et, jp):
                h, t2 = u % HPG, u // HPG
                ps = pssc.tile([P, 2 * NT], F32, tag="ps", name="pssc")
                for i in range(2):
                    so = 2 * jp + i
                    nc.tensor.matmul(
                        ps[:, i * NT:(i + 1) * NT],
                        kt_sb[_hp(h), h // 2, so * P:(so + 1) * P],
                        qt_sb[_hp(h), h // 2, t2 * NT:(t2 + 1) * NT],
                        start=True, stop=True)
                nc.scalar.activation(
                    et[:, 2 * jp:2 * jp + 2, :].rearrange("p a b -> p (a b)"),
                    ps, ACTF.Exp, scale=1.0 / 32.0)

            def mask_unit(u, et):
                t2 = u // HPG
                nc.vector.tensor_tensor(
                    et, et, mask_sb[:, :, t2 * NT:(t2 + 1) * NT], ALU.mult)

            def dproj_pair(tm):
                # both c2 halves in one 1024-wide pssc tile (free in the
                # tail), single wide eviction + single out DMA
                ps = pssc.tile([P, 2 * NT], F32, tag="ps", name="psdp")
                for c2 in range(2):
                    for kp in range(KP):
                        nc.tensor.matmul(
                            ps[:, c2 * NT:(c2 + 1) * NT],
                            ot_sb[:, kp, tm * P:(tm + 1) * P],
                            wo_sb[:, kp, c2 * NT:(c2 + 1) * NT],
                            start=(kp == 0), stop=(kp == KP - 1))
                o_sb = outp.tile([P, 2 * NT], BF16, tag="ob")
                nc.scalar.activation(o_sb, ps, ACTF.Copy)
                nc.sync.dma_start(out[tm * P:(tm + 1) * P, :], o_sb)

            # ---- software pipeline ----
            # 64 filler chunks, one per scores-group gap:
            #   units 0-5 gaps: q/k projection m-groups 1-3 (4 chunks each)
            #   units 6-7 gaps: V' projection (4 half-chunks per unit)
            #   units 8-15 gaps: PV of units 0-7 (lag 8; frees et for reuse)
            et_q = {}
            fillers = []
            for mg in range(1, MQ):
                for (w, s_, d_) in ((wq_sb, xT_sb, qt_sb),
                                    (wk_sb, ctxT_sb, kt_sb)):
                    for piece in range(4):
                        fillers.append(
                            (lambda w=w, s_=s_, d_=d_, mg=mg, piece=piece:
                             proj_chunk(w, s_, d_, mg, piece)))
            for jp in range(4):
                for half in range(2):
                    fillers.append(
                        (lambda jp=jp, half=half: vproj_chunk(jp, half)))
            for u in range(8):
                for piece in range(4):
                    fillers.append(
                        (lambda u=u, piece=piece: pv_chunk(u, piece)))

            warmup()
            for piece in range(4):
                proj_chunk(wq_sb, xT_sb, qt_sb, 0, piece)
            for piece in range(4):
                proj_chunk(wk_sb, ctxT_sb, kt_sb, 0, piece)

            fi = 0
            for u in range(NU):
                et_q[u] = etp.tile([P, SO, NT], BF16, tag="et",
                                   name=f"et{u}")
                for jp in range(4):
                    scores_group(u, et_q[u], jp)
                    fillers[fi]()
                    fi += 1
                mask_unit(u, et_q[u])
            for u in range(8, 16):
                for piece in range(4):
                    pv_chunk(u, piece)
                if u < 12:
                    dproj_pair(u - 8)   # t2=0 rows, deps done in the middle
            for tm in range(4, 8):
                dproj_pair(tm)
    nc.compile()
    return nc


def _get_program():
    global _CACHED_NC
    if _CACHED_NC is None:
        _CACHED_NC = _build_program()
    return _CACHED_NC


def kernel(x, context, attn_mask, Wq, Wk, Wv, Wo):
    x = np.asarray(x, dtype=np.float32)
    context = np.asarray(context, dtype=np.float32)
    attn_mask = np.asarray(attn_mask)
    Wq = np.asarray(Wq, dtype=np.float32)
    Wk = np.asarray(Wk, dtype=np.float32)
    Wv = np.asarray(Wv, dtype=np.float32)
    Wo = np.asarray(Wo, dtype=np.float32)

    nc = _get_program()
    bf = ml_dtypes.bfloat16
    in_maps = []
    for i in range(NCORES):
        b, g = i // 2, i % 2
        cs = slice(g * CG, (g + 1) * CG)
        def tile_rows(a, n):
            # [n*128, m] -> [128, n, m] with rows r = k*128+p at [p, k]
            return np.ascontiguousarray(
                a.reshape(n, P, -1).transpose(1, 0, 2)).astype(bf)
        in_maps.append({
            "xT": tile_rows(x[b].T, KO),
            "ctxT": tile_rows(context[b].T, KO),
            "maskT": tile_rows(attn_mask[b, 0].T.astype(np.float32), SO),
            "wq": np.stack([tile_rows(Wq[:, cs][:, m * P:(m + 1) * P], KO)
                            for m in range(MQ)]),
            "wk": np.stack([tile_rows(Wk[:, cs][:, m * P:(m + 1) * P], KO)
                            for m in range(MQ)]),
            "wv": tile_rows(Wv[:, cs], KO),
            "wo": tile_rows(Wo[cs, :], KP),
        })

    profile = os.environ.get("KERNEL_PROFILE", "0") == "1"
    if profile:
        _ensure_ntff_hook()
    res = run_bass_kernel_spmd(
        nc, in_maps, list(range(NCORES)),
        trace=profile, trace_cores=[0] if profile else None)
    if profile:
        kernel.last_exec_time_ns = res.exec_time_ns
        kernel.last_trace = res.instructions_and_trace

    out = np.empty((B, T, C), dtype=np.float32)
    for b in range(B):
        out[b] = (res.results[2 * b]["out"].astype(np.float32)
                  + res.results[2 * b + 1]["out"].astype(np.float32))
    return out
